# revision 23
# baseline (speedup 1.0000x reference)
"""Trainium2 kernel for nn_Block1SyntaxEngine_85959475462663
(6-layer dense transformer, B=2 T=1024 D=1024 H=16 DFF=2048, fp32 ref).

Distribution: 2-way data-parallel over batch (core groups [0-3], [4-7]) x
4-way Megatron tensor-parallel inside each group (4 heads + 512 d_ff columns
per core).

This version keeps the residual stream FEATURE-MAJOR ([D, T], fp32) in SBUF
for the whole kernel, which removes every activation transpose (the previous
version bounced activations through DRAM twice per layernorm).  LayerNorm
statistics are computed with ones-vector matmuls over the partition axis
(fp32r, full rate at N=512) and broadcast back with K=1 matmuls; attn@v is
computed as v_ext^T @ E so the attention output lands feature-major with
N=512 matmuls; softmax normalization is a reciprocal row broadcast by a
ones-matmul folded into the PSUM evacuation.  Embedding gather + positional
add happen on the host (staged input), output is written feature-major and
transposed on the host.  fp16 matmul inputs (fp32 PSUM), LayerNorm scale/bias
folded into the following weights on the host, two fp16 AllReduces per layer
split into two 512-token chunks for overlap.

Self-contained: only needs numpy/jax/concourse (the trn_rl_repo toolchain
on sys.path) and 8 visible neuron cores.
"""
import contextlib
import time

import numpy as np

import concourse.bass as bass
import concourse.mybir as mybir
import concourse.tile as tile
from concourse import bacc

P = 128
B, T, D, H, L, V = 2, 1024, 1024, 16, 6, 32000
DH = D // H            # 64
DFF = 2 * D            # 2048
NCORES = 8
NG = 4                 # tensor-parallel degree (cores per group)
HR = H // NG           # heads per core (4)
FR = DFF // NG         # ffn columns per core (512)
TT = T // P            # token tiles (8)
KT = D // P            # contraction tiles over D (8)
CW = 512               # token-chunk width (2 chunks per layer stage)
NC = T // CW           # 2 chunks

f16 = mybir.dt.float16
f32 = mybir.dt.float32
f32r = mybir.dt.float32r
AF = mybir.ActivationFunctionType
ALU = mybir.AluOpType
EPS = 1e-5
SIM_GELU_SUBST = False   # True: use Sigmoid instead of Gelu (sim lacks Gelu)
NO_COMM = False          # True: replace AllReduce with local copy (debug)
MERGE_AR = False         # True: one 2MB AllReduce per site instead of 2x1MB
GROUPS = [[0, 1, 2, 3], [4, 5, 6, 7]]


def build_nc():
    nc = bacc.Bacc()
    dp = dict(
        x0T=nc.declare_dram_parameter("x0T", [D, T], f16, isOutput=False),
        wqk=nc.declare_dram_parameter("wqk", [L, D, 4, P], f16, isOutput=False),
        bqk=nc.declare_dram_parameter("bqk", [L, 4, P], f32, isOutput=False),
        wv=nc.declare_dram_parameter("wv", [L, D, HR * DH], f16, isOutput=False),
        bv=nc.declare_dram_parameter("bv", [L, HR * DH], f32, isOutput=False),
        wout=nc.declare_dram_parameter("wout", [L, HR * DH, D], f16, isOutput=False),
        w1=nc.declare_dram_parameter("w1", [L, D, FR], f16, isOutput=False),
        b1=nc.declare_dram_parameter("b1", [L, 4, P], f32, isOutput=False),
        w2=nc.declare_dram_parameter("w2", [L, FR, D], f16, isOutput=False),
        lnf=nc.declare_dram_parameter("lnf", [2, D], f32, isOutput=False),
        mask=nc.declare_dram_parameter("mask", [P, P], f16, isOutput=False),
        out=nc.declare_dram_parameter("out", [D, T], f32, isOutput=True),
    )
    with tile.TileContext(nc) as tc:
        _body(nc, tc, dp)
    nc.finalize()
    return nc


def _body(nc, tc, dp):
    ctx = contextlib.ExitStack()
    with ctx:
        # ---- SBUF pools ----
        xp = ctx.enter_context(tc.tile_pool(name="xp", bufs=1))        # x resid
        cst = ctx.enter_context(tc.tile_pool(name="cst", bufs=1))      # consts
        wp = ctx.enter_context(tc.tile_pool(name="wp", bufs=1))        # weights
        xcp = ctx.enter_context(tc.tile_pool(name="xcp", bufs=2))      # ln'd act
        sqp = ctx.enter_context(tc.tile_pool(name="sqp", bufs=1))      # x^2
        qkp = ctx.enter_context(tc.tile_pool(name="qkp", bufs=1))      # q/k feat
        vp = ctx.enter_context(tc.tile_pool(name="vp", bufs=1))        # v + ones
        ep = ctx.enter_context(tc.tile_pool(name="ep", bufs=2))        # exp(scores)
        otp = ctx.enter_context(tc.tile_pool(name="otp", bufs=1))      # attn out
        hp = ctx.enter_context(tc.tile_pool(name="hp", bufs=1))        # gelu(h1)
        stg = ctx.enter_context(tc.tile_pool(name="stg", bufs=2))      # AR staging
        rws = ctx.enter_context(tc.tile_pool(name="rws", bufs=2))      # [1,CW] rows
        rnp = ctx.enter_context(tc.tile_pool(name="rnp", bufs=2))      # rn bcast
        tmp = ctx.enter_context(tc.tile_pool(name="tmp", bufs=3))      # [128,CW]
        dmp = ctx.enter_context(tc.tile_pool(name="dmp", bufs=5, space="DRAM"))
        # ---- PSUM pools (8 banks total) ----
        ps512 = ctx.enter_context(tc.tile_pool(name="ps512", bufs=4, space="PSUM"))
        psav = ctx.enter_context(tc.tile_pool(name="psav", bufs=2, space="PSUM"))
        psst = ctx.enter_context(tc.tile_pool(name="psst", bufs=1, space="PSUM"))
        psln = ctx.enter_context(tc.tile_pool(name="psln", bufs=1, space="PSUM"))

        # ---- constants ----
        mask16 = cst.tile([P, P], f16)
        nc.sync.dma_start(mask16[:], dp["mask"][:])
        lnf_t = cst.tile([P, KT, 2], f32)
        for s in range(2):
            nc.sync.dma_start(lnf_t[:, :, s],
                              dp["lnf"][s].rearrange("(kt p) -> p kt", p=P))
        ones16c = cst.tile([P, 1], f16)
        nc.vector.memset(ones16c[:], 1.0)
        ones16r = cst.tile([1, P], f16)
        nc.vector.memset(ones16r[:], 1.0)
        eps_r = cst.tile([1, 1], f32)
        nc.vector.memset(eps_r[:], EPS)

        # ---- residual (feature-major) ----
        x = xp.tile([P, KT, T], f16)
        nc.sync.dma_start(x[:], dp["x0T"].rearrange("(kt p) t -> p kt t", p=P))

        # v_ext: [t-part, jt, head, DH+1]; trailing col stays 1.0 (row sums)
        v_ext = vp.tile([P, TT, HR, DH + 1], f16, tag="vext")
        nc.vector.memset(v_ext[:], 0.0)
        nc.vector.memset(v_ext[:, :, :, DH], 1.0)
        zback = None
        if NO_COMM == "fast":
            zback = vp.tile([P, KT, CW], f16, tag="zback")
            nc.vector.memset(zback[:], 0.0)

        def ln_stats(c):
            """LN stats for token-chunk c: PE ones-matmul reductions over the
            feature (partition) axis, rows broadcast back via K=1 matmuls.
            Returns the [P,CW] f16 (-mean, 1/std) broadcast tiles."""
            cs = slice(c * CW, (c + 1) * CW)
            p_sum = psst.tile([1, CW], f32, tag="st")
            for kt in range(KT):
                nc.tensor.matmul(p_sum[:], ones16c[:], x[:, kt, cs],
                                 start=(kt == 0), stop=(kt == KT - 1))
            nm16 = rws.tile([1, CW], f16, tag="nm16")
            with nc.allow_low_precision(reason="f16 LN rows feed f16 matmuls"):
                nc.vector.tensor_scalar_mul(nm16[:], p_sum[:], -1.0 / D)
            p_sq = psst.tile([1, CW], f32, tag="st")
            for kt in range(KT):
                xsq = sqp.tile([P, CW], f16, tag="xsq", bufs=3)
                nc.scalar.activation(xsq[:], x[:, kt, cs], AF.Square)
                nc.tensor.matmul(p_sq[:], ones16c[:], xsq[:],
                                 start=(kt == 0), stop=(kt == KT - 1))
            ra = rws.tile([1, CW], f16, tag="rowA")
            rb = rws.tile([1, CW], f16, tag="rowB")
            with nc.allow_low_precision(reason="f16 LN rows feed f16 matmuls"):
                nc.vector.tensor_scalar_mul(ra[:], p_sq[:], 1.0 / D)
                nc.vector.tensor_mul(rb[:], nm16[:], nm16[:])
                nc.vector.tensor_sub(ra[:], ra[:], rb[:])
                nc.scalar.activation(rb[:], ra[:], AF.Sqrt, bias=eps_r[:])
                rstd16 = rws.tile([1, CW], f16, tag="rstd")
                nc.vector.reciprocal(rstd16[:], rb[:])
            p_nm = psln.tile([P, CW], f32, tag="lnb")
            nc.tensor.matmul(p_nm[:], ones16r[:], nm16[:], start=True, stop=True)
            nm_b = rnp.tile([P, CW], f16, tag="nm_b")
            nc.scalar.activation(nm_b[:], p_nm[:], AF.Copy)
            p_rs = psln.tile([P, CW], f32, tag="lnb")
            nc.tensor.matmul(p_rs[:], ones16r[:], rstd16[:], start=True, stop=True)
            rs_b = rnp.tile([P, CW], f16, tag="rs_b")
            nc.scalar.activation(rs_b[:], p_rs[:], AF.Copy)
            return nm_b, rs_b

        def ln_norm(c, rows, dst16):
            """Apply (x - mean) * rstd for chunk c into dst16; dst16 None =>
            final LN with lnf scale/bias + output DMA."""
            cs = slice(c * CW, (c + 1) * CW)
            nm_b, rs_b = rows
            for kt in range(KT):
                t = tmp.tile([P, CW], f16, tag="lnt")
                nc.vector.tensor_add(t[:], x[:, kt, cs], nm_b[:])
                if dst16 is not None:
                    nc.vector.tensor_mul(dst16[:, kt, cs], t[:], rs_b[:])
                else:
                    z = tmp.tile([P, CW], f32, tag="lnz")
                    nc.vector.tensor_mul(z[:], t[:], rs_b[:])
                    nc.vector.tensor_scalar(
                        z[:], z[:], lnf_t[:, kt, 0:1], lnf_t[:, kt, 1:2],
                        ALU.mult, ALU.add)
                    nc.sync.dma_start(dp["out"][kt * P:(kt + 1) * P, cs], z[:])

        def layernorm(c, dst16):
            ln_norm(c, ln_stats(c), dst16)

        ARW = T if MERGE_AR else CW  # tokens per AllReduce call

        def ar_launch(part16):
            """Stage the [P, KT, ARW] f16 partial to DRAM and AllReduce it."""
            ar_i = dmp.tile([D, ARW], f16, tag="ar_in")
            ar_o = dmp.tile([D, ARW], f16, tag="ar_out")
            nc.sync.dma_start(ar_i[:].rearrange("(o p) t -> p o t", p=P), part16[:])
            if NO_COMM == "fast":
                pass
            elif NO_COMM:
                nc.sync.dma_start(ar_o[:], ar_i[:])
            else:
                nc.gpsimd.collective_compute(
                    "AllReduce", ALU.add, replica_groups=GROUPS,
                    ins=[ar_i[:]], outs=[ar_o[:]],
                )
            return ar_o

        def ar_consume(ar_o, c):
            """Read the AllReduce result back and add into x chunk c
            (c=None under MERGE_AR: both chunks)."""
            cs = slice(0, T) if c is None else slice(c * CW, (c + 1) * CW)
            if NO_COMM == "fast":
                back = zback
            else:
                back = stg.tile([P, KT, ARW], f16, tag="part")
                nc.sync.dma_start(back[:],
                                  ar_o[:].rearrange("(o p) t -> p o t", p=P))
            for kt in range(KT):
                nc.vector.tensor_add(x[:, kt, cs], x[:, kt, cs], back[:, kt, :])

        # ================= layers =================
        # h_ff1_prev: the previous layer's chunk-1 ffn AllReduce, consumed
        # lazily inside the next layer (after AR_at0 launches) so its DVE
        # adds never block the next layer's chunk-0 LN/attention chain.
        h_ff1_prev = None
        for l in range(L):
            wqk_t = wp.tile([P, KT, 4, P], f16, tag="wqk")
            nc.sync.dma_start(wqk_t[:], dp["wqk"][l].rearrange("(kt p) m n -> p kt m n", p=P))
            w1_t = wp.tile([P, KT, FR], f16, tag="w1")
            nc.sync.dma_start(w1_t[:], dp["w1"][l].rearrange("(kt p) n -> p kt n", p=P))
            w2_t = wp.tile([P, 4, D], f16, tag="w2")
            nc.sync.dma_start(w2_t[:], dp["w2"][l].rearrange("(kt p) n -> p kt n", p=P))
            wv_t = wp.tile([P, KT, HR * DH], f16, tag="wv")
            nc.sync.dma_start(wv_t[:], dp["wv"][l].rearrange("(kt p) n -> p kt n", p=P))
            wout_t = wp.tile([P, 2, D], f16, tag="wout")
            nc.sync.dma_start(wout_t[:], dp["wout"][l].rearrange("(kt p) n -> p kt n", p=P))
            bqk_t = wp.tile([P, 4], f32, tag="bqk")
            nc.sync.dma_start(bqk_t[:], dp["bqk"][l].rearrange("m p -> p m"))
            bv_t = wp.tile([P, HR * DH], f32, tag="bv")
            nc.sync.dma_start(bv_t[:], dp["bv"][l, None, :].to_broadcast((P, HR * DH)))
            b1_t = wp.tile([P, 4], f32, tag="b1")
            nc.sync.dma_start(b1_t[:], dp["b1"][l].rearrange("m p -> p m"))

            scale = float(1.0 / np.sqrt(DH))
            xc = xcp.tile([P, KT, T], f16, tag="xc")
            xc2 = xcp.tile([P, KT, T], f16, tag="xc")
            qkT = qkp.tile([P, 4, T], f16, tag="qkT")
            oT = otp.tile([P, 2, T], f16, tag="oT")
            h1g = hp.tile([P, 4, T], f16, tag="h1g")

            def qkv_chunk(c):
                """q/k projections for chunk c + v for token tiles of c."""
                cs = slice(c * CW, (c + 1) * CW)
                for mt in range(4):
                    pt = ps512.tile([P, CW], f32, tag="mm512")
                    for kt in range(KT):
                        nc.tensor.matmul(pt[:], wqk_t[:, kt, mt, :], xc[:, kt, cs],
                                         start=(kt == 0), stop=(kt == KT - 1))
                    nc.scalar.activation(qkT[:, mt, cs], pt[:], AF.Identity,
                                         bias=bqk_t[:, mt, None])
                for tt in range(4 * c, 4 * c + 4):
                    pv = ps512.tile([P, HR * DH], f32, tag="mm512")
                    for kt in range(KT):
                        nc.tensor.matmul(pv[:], xc[:, kt, tt * P:(tt + 1) * P],
                                         wv_t[:, kt, :],
                                         start=(kt == 0), stop=(kt == KT - 1))
                    nc.vector.tensor_add(v_ext[:, tt, :, 0:DH],
                                         pv[:].rearrange("p (h d) -> p h d", h=HR),
                                         bv_t[:].rearrange("p (h d) -> p h d", h=HR))

            def _kq(h):
                rsl = slice(DH * (h % 2), DH * (h % 2) + DH)
                return qkT[rsl, 2 + h // 2, :], qkT[rsl, h // 2, :]

            def scores_c0_pair(ha):
                """E for heads ha, ha+1, chunk 0.  The two heads' k/q live on
                partition strips 0-63 / 64-127, so their matmuls land in
                distinct PE row-groups and run concurrently when adjacent."""
                (ka, qa), (kb, qb) = _kq(ha), _kq(ha + 1)
                Ea = ep.tile([P, 4, CW], f16, tag="E0", bufs=2)
                Eb = ep.tile([P, 4, CW], f16, tag="E0", bufs=2)
                for jt in range(4):
                    vc = slice(jt * P, CW)
                    js = slice(jt * P, (jt + 1) * P)
                    pea = ps512.tile([P, CW], f32, tag="mm512")
                    nc.tensor.matmul(pea[:, vc], ka[:, js], qa[:, vc],
                                     start=True, stop=True)
                    peb = ps512.tile([P, CW], f32, tag="mm512")
                    nc.tensor.matmul(peb[:, vc], kb[:, js], qb[:, vc],
                                     start=True, stop=True)
                    for E, pe in ((Ea, pea), (Eb, peb)):
                        nc.scalar.activation(E[:, jt, vc], pe[:, vc], AF.Exp,
                                             scale=scale)
                        nc.vector.tensor_mul(E[:, jt, js], E[:, jt, js],
                                             mask16[:])
                return Ea, Eb

            def scores_c1_pair(ha):
                """E for heads ha, ha+1, chunk 1 (j-tiles 0-7)."""
                (ka, qa), (kb, qb) = _kq(ha), _kq(ha + 1)
                Ea = ep.tile([P, TT, CW], f16, tag="E1", bufs=2)
                Eb = ep.tile([P, TT, CW], f16, tag="E1", bufs=2)
                for jt in range(TT):
                    vc = slice(max(0, (jt - 4) * P), CW)
                    js = slice(jt * P, (jt + 1) * P)
                    pea = ps512.tile([P, CW], f32, tag="mm512")
                    nc.tensor.matmul(pea[:, vc], ka[:, js],
                                     qa[:, CW + vc.start:T], start=True,
                                     stop=True)
                    peb = ps512.tile([P, CW], f32, tag="mm512")
                    nc.tensor.matmul(peb[:, vc], kb[:, js],
                                     qb[:, CW + vc.start:T], start=True,
                                     stop=True)
                    for E, pe in ((Ea, pea), (Eb, peb)):
                        nc.scalar.activation(E[:, jt, vc], pe[:, vc], AF.Exp,
                                             scale=scale)
                        if jt >= 4:
                            lc = (jt - 4) * P
                            nc.vector.tensor_mul(E[:, jt, lc:lc + P],
                                                 E[:, jt, lc:lc + P],
                                                 mask16[:])
                return Ea, Eb

            def av(h, c, Ec):
                cs = slice(c * CW, (c + 1) * CW)
                njt = 4 if c == 0 else TT
                po = psav.tile([DH + 1, CW], f32, tag="mmav")
                for jt in range(njt):
                    vc = slice(max(0, (jt - 4 if c else jt)) * P, CW)
                    nc.tensor.matmul(po[:, vc], v_ext[:, jt, h, :],
                                     Ec[:, jt, vc],
                                     start=(jt == 0), stop=(jt == njt - 1))
                rn16 = rws.tile([1, CW], f16, tag="rn", bufs=4)
                with nc.allow_low_precision(reason="f16 softmax 1/sum"):
                    nc.vector.reciprocal(rn16[:], po[DH:DH + 1, :])
                prn = psln.tile([DH, CW], f32, tag="lnb")
                nc.tensor.matmul(prn[:], ones16r[0:1, 0:DH], rn16[:],
                                 start=True, stop=True)
                rnb = rnp.tile([DH, CW], f16, tag="rnb16")
                nc.scalar.activation(rnb[:], prn[:], AF.Copy)
                rsl = slice(DH * (h % 2), DH * (h % 2) + DH)
                nc.vector.tensor_mul(oT[rsl, h // 2, cs], po[0:DH, :], rnb[:])

            def wout_chunk(c, part16):
                cs = slice(c * CW, (c + 1) * CW)
                for nt in range(KT):
                    pw = ps512.tile([P, CW], f32, tag="mm512")
                    for pr in range(2):
                        nc.tensor.matmul(pw[:], wout_t[:, pr, nt * P:(nt + 1) * P],
                                         oT[:, pr, cs], start=(pr == 0),
                                         stop=(pr == 1))
                    nc.scalar.activation(part16[:, nt, :], pw[:], AF.Copy)

            def ffn_chunk(c, part16):
                cs = slice(c * CW, (c + 1) * CW)
                ls = cs if MERGE_AR else slice(0, CW)
                for ft in range(4):
                    pf = ps512.tile([P, CW], f32, tag="mm512")
                    for kt in range(KT):
                        nc.tensor.matmul(pf[:], w1_t[:, kt, ft * P:(ft + 1) * P],
                                         xc2[:, kt, cs],
                                         start=(kt == 0), stop=(kt == KT - 1))
                    nc.scalar.activation(h1g[:, ft, cs], pf[:],
                                         AF.Sigmoid if SIM_GELU_SUBST else AF.Gelu,
                                         bias=b1_t[:, ft, None])
                for nt in range(KT):
                    pw = ps512.tile([P, CW], f32, tag="mm512")
                    for ft in range(4):
                        nc.tensor.matmul(pw[:], w2_t[:, ft, nt * P:(nt + 1) * P],
                                         h1g[:, ft, cs], start=(ft == 0),
                                         stop=(ft == 3))
                    nc.scalar.activation(part16[:, nt, ls], pw[:], AF.Copy)

            # ---- skewed two-chunk schedule: chunk 1's attention fills ----
            # ---- chunk 0's attention-AR latency, FFN fills the rest.  ----
            def attention(c):
                if c == 0:
                    E0s = scores_c0_pair(0) + scores_c0_pair(2)
                    for h in range(HR):
                        av(h, 0, E0s[h])
                else:
                    Ea, Eb = scores_c1_pair(0)
                    av(0, 1, Ea)
                    Ec, Ed = scores_c1_pair(2)
                    av(1, 1, Eb)
                    av(2, 1, Ec)
                    av(3, 1, Ed)

            r0 = ln_stats(0)
            ln_norm(0, r0, xc)
            qkv_chunk(0)
            attention(0)
            p0 = stg.tile([P, KT, CW], f16, tag="part")
            wout_chunk(0, p0)
            h_at0 = ar_launch(p0)
            if h_ff1_prev is not None:
                ar_consume(h_ff1_prev, 1)
            r1 = ln_stats(1)
            ln_norm(1, r1, xc)
            qkv_chunk(1)
            attention(1)
            p1 = stg.tile([P, KT, CW], f16, tag="part")
            wout_chunk(1, p1)
            h_at1 = ar_launch(p1)
            ar_consume(h_at0, 0)
            r2 = ln_stats(0)
            ln_norm(0, r2, xc2)
            p2 = stg.tile([P, KT, CW], f16, tag="part")
            ffn_chunk(0, p2)
            h_ff0 = ar_launch(p2)
            ar_consume(h_at1, 1)
            r3 = ln_stats(1)
            ln_norm(1, r3, xc2)
            p3 = stg.tile([P, KT, CW], f16, tag="part")
            ffn_chunk(1, p3)
            h_ff1 = ar_launch(p3)
            ar_consume(h_ff0, 0)
            h_ff1_prev = h_ff1

        # ---- final layernorm + output ----
        layernorm(0, None)
        ar_consume(h_ff1_prev, 1)
        layernorm(1, None)


# ======================= host side =======================

def _prep_inputs(input_ids, token_emb, pos_emb, ln1_s, ln1_b, Wqkv, Wout,
                 ln2_s, ln2_b, W1, W2, lnf_s, lnf_b):
    ids_np = np.asarray(input_ids)
    emb = np.asarray(token_emb, np.float32)
    pos = np.asarray(pos_emb, np.float32)
    # E[j, q] is valid where j <= q: upper triangle in (j=partition, q=free)
    mask_np = np.triu(np.ones((P, P), np.float32)).astype(np.float16)
    Wqkv64 = np.asarray(Wqkv, np.float64)
    W164 = np.asarray(W1, np.float64)
    Wqkv_f = Wqkv64 * np.asarray(ln1_s, np.float64)[:, :, None]
    bqkv_f = np.einsum("ld,ldn->ln", np.asarray(ln1_b, np.float64), Wqkv64)
    W1_f = W164 * np.asarray(ln2_s, np.float64)[:, :, None]
    b1_f = np.einsum("ld,ldn->ln", np.asarray(ln2_b, np.float64), W164)
    lnf_sb = np.stack([np.asarray(lnf_s, np.float32),
                       np.asarray(lnf_b, np.float32)])
    x0T = [np.ascontiguousarray((emb[ids_np[g]] + pos).T).astype(np.float16)
           for g in range(B)]

    in_maps = []
    for core in range(NCORES):
        g, r = divmod(core, NG)
        heads = [HR * r + i for i in range(HR)]
        wqk_np = np.empty((L, D, 4, P), np.float16)
        bqk_np = np.empty((L, 4, P), np.float32)
        for pr in range(2):
            h0, h1 = heads[2 * pr], heads[2 * pr + 1]
            qcols = np.r_[DH * h0:DH * h0 + DH, DH * h1:DH * h1 + DH]
            kcols = D + qcols
            wqk_np[:, :, pr, :] = Wqkv_f[:, :, qcols].astype(np.float16)
            wqk_np[:, :, 2 + pr, :] = Wqkv_f[:, :, kcols].astype(np.float16)
            bqk_np[:, pr, :] = bqkv_f[:, qcols].astype(np.float32)
            bqk_np[:, 2 + pr, :] = bqkv_f[:, kcols].astype(np.float32)
        vcols = np.r_[tuple(np.arange(2 * D + DH * h, 2 * D + DH * h + DH)
                            for h in heads)]
        orows = np.r_[tuple(np.arange(DH * h, DH * h + DH) for h in heads)]
        in_maps.append(dict(
            x0T=x0T[g],
            wqk=wqk_np,
            bqk=bqk_np,
            wv=Wqkv_f[:, :, vcols].astype(np.float16),
            bv=bqkv_f[:, vcols].astype(np.float32),
            wout=np.asarray(Wout, np.float16)[:, orows, :],
            w1=W1_f[:, :, FR * r:FR * (r + 1)].astype(np.float16),
            b1=b1_f[:, FR * r:FR * (r + 1)].astype(np.float32).reshape(L, 4, P),
            w2=np.asarray(W2, np.float16)[:, FR * r:FR * (r + 1), :],
            lnf=lnf_sb, mask=mask_np,
        ))
    return in_maps


# ---------- compile-once / run-many PJRT runner (vendored) ----------

class SpmdRunner:
    def __init__(self, nc, n_cores=8):
        import jax
        from jax.sharding import Mesh, PartitionSpec
        from jax.experimental.shard_map import shard_map
        from concourse.bass2jax import (
            _bass_exec_p, install_neuronx_cc_hook, partition_id_tensor)
        self.jax = jax
        self.PartitionSpec = PartitionSpec
        install_neuronx_cc_hook()
        if not nc.is_finalized():
            nc.finalize()
        self.n_cores = n_cores
        partition_name = (
            nc.partition_id_tensor.name if nc.partition_id_tensor else None)
        in_names, out_names, out_avals, zero_outs = [], [], [], []
        for alloc in nc.m.functions[0].allocations:
            if not isinstance(alloc, mybir.MemoryLocationSet):
                continue
            name = alloc.memorylocations[0].name
            if alloc.kind == "ExternalInput":
                if name != partition_name:
                    in_names.append(name)
            elif alloc.kind == "ExternalOutput":
                out_names.append(name)
                shape = tuple(alloc.tensor_shape)
                dtype = mybir.dt.np(alloc.dtype)
                out_avals.append(jax.core.ShapedArray(shape, dtype))
                zero_outs.append(np.zeros(shape, dtype))
        self.in_names, self.out_names = in_names, out_names
        self.out_avals, self.zero_outs = out_avals, zero_outs
        n_params, n_outs = len(in_names), len(out_avals)
        self.n_params = n_params
        all_in = in_names + out_names + (
            [partition_name] if partition_name else [])
        donate = tuple(range(n_params, n_params + n_outs))

        def _b(*args):
            ops = list(args)
            if partition_name:
                ops.append(partition_id_tensor())
            return tuple(_bass_exec_p.bind(
                *ops, out_avals=tuple(out_avals), in_names=tuple(all_in),
                out_names=tuple(out_names), lowering_input_output_aliases=(),
                sim_require_finite=True, sim_require_nnan=True, nc=nc))

        devices = jax.devices()[:n_cores]
        self.mesh = Mesh(np.asarray(devices), ("core",))
        specs = (PartitionSpec("core"),)
        self.sharded = jax.jit(
            shard_map(_b, mesh=self.mesh,
                      in_specs=specs * (n_params + n_outs),
                      out_specs=specs * len(out_names), check_rep=False),
            donate_argnums=donate, keep_unused=True)
        self._dev_inputs = None

    def _zeros(self):
        return [np.zeros((self.n_cores * z.shape[0], *z.shape[1:]), z.dtype)
                for z in self.zero_outs]

    def stage_inputs(self, in_maps):
        jax, PS = self.jax, self.PartitionSpec
        per_core = [[np.asarray(m[n]) for n in self.in_names] for m in in_maps]
        concat = [np.concatenate([per_core[c][i] for c in range(self.n_cores)],
                                 axis=0) for i in range(self.n_params)]
        sh = jax.sharding.NamedSharding(self.mesh, PS("core"))
        self._dev_inputs = [jax.device_put(a, sh) for a in concat]
        for a in self._dev_inputs:
            a.block_until_ready()

    def run(self, in_maps=None):
        if in_maps is not None:
            self.stage_inputs(in_maps)
        outs = self.sharded(*self._dev_inputs, *self._zeros())
        out_np = [np.asarray(a) for a in outs]
        return [{n: out_np[i].reshape(self.n_cores, *self.out_avals[i].shape)[c]
                 for i, n in enumerate(self.out_names)}
                for c in range(self.n_cores)]

    def time_exec(self, iters=8, warmup=2):
        jax, PS = self.jax, self.PartitionSpec
        sh = jax.sharding.NamedSharding(self.mesh, PS("core"))
        zsets = [[jax.device_put(z, sh) for z in self._zeros()]
                 for _ in range(warmup + iters)]
        for zs in zsets:
            for z in zs:
                z.block_until_ready()
        outs = []
        for i in range(warmup):
            outs.append(self.sharded(*self._dev_inputs, *zsets[i]))
        for o in outs[-1]:
            o.block_until_ready()
        t0 = time.perf_counter()
        outs = []
        for i in range(iters):
            outs.append(self.sharded(*self._dev_inputs, *zsets[warmup + i]))
        for o in outs[-1]:
            o.block_until_ready()
        return (time.perf_counter() - t0) / iters


_RUNNER = None


def get_runner():
    global _RUNNER
    if _RUNNER is None:
        _RUNNER = SpmdRunner(build_nc(), NCORES)
    return _RUNNER


def kernel(**inputs) -> np.ndarray:
    in_maps = _prep_inputs(**{k: np.asarray(v) for k, v in inputs.items()})
    res = get_runner().run(in_maps)
    out = np.empty((B, T, D), np.float32)
    out[0] = res[0]["out"].T
    out[1] = res[NG]["out"].T
    return out

